# revision 1
# baseline (speedup 1.0000x reference)
"""Trainium2 Bass kernel for ConvolutionFeatureProcessor.

Math (per item, matching the jax reference):
  h[t]   = relu(b1 + sum_k x[t+k] @ w1k^T)          t in [0, T-2)
  pooled = (1/(L-2)) * sum_{t<L-2} h[t]             (masked mean)
  p2     = W2 @ pooled + b2      (td2 is linear -> commutes with the mean)
  out    = MLP(p2)               (64 -> 256 -> 256 -> 512)

Device strategy (8 cores, pure data parallel over the batch):
  - 16 items/core, processed in pairs packed on the 128 partitions.
  - x is cast f32->bf16 during the HBM load (SWDGE), then transposed to
    [d, t] layout with the xbar DMA-transpose (2 items side by side give
    the required 128-wide free dim).
  - conv: 3 shifted matmuls with block-diag(w_k^T, w_k^T) stationary
    + a 4th accumulating "mask matmul" that adds -30000 to columns
    t >= L-2 (per item) so relu zeroes invalid frames exactly.
  - ACT does bias+relu and the temporal sum in one op (accum_out).
  - Tiny MLP in f32 on PE at the end; output transposed back via PE.
"""

import numpy as np

B, T, D, OUT = 128, 4096, 64, 512
NCORES = 8
BS = B // NCORES  # items per core
NPAIR = BS // 2
NEG = -30000.0
TC = 512  # conv chunk (free dim per matmul)
NCHUNK = T // TC  # 8; last chunk covers 510 valid outputs

_CACHE = {}


def _build(stage=99, repeat=1, loop_n=0, bufs=(2, 2, 2, 3)):
    import concourse.bacc as bacc
    import concourse.mybir as mybir
    import concourse.tile as tile
    from concourse.masks import make_identity

    f32 = mybir.dt.float32
    bf16 = mybir.dt.bfloat16
    i32 = mybir.dt.int32
    AX = mybir.AxisListType
    OP = mybir.AluOpType
    AF = mybir.ActivationFunctionType

    nc = bacc.Bacc("TRN2", target_bir_lowering=False, debug=False)

    # 2D shape: the axon-side sharding jit chokes on 3D dynamic-slices
    x_d = nc.dram_tensor("x", [BS, T * D], f32, kind="ExternalInput").ap()
    len_d = nc.dram_tensor("lengths", [BS, 1], i32, kind="ExternalInput").ap()
    W1_d = nc.dram_tensor("W1", [D, 3 * D], f32, kind="ExternalInput").ap()
    b1_d = nc.dram_tensor("b1", [D, 1], f32, kind="ExternalInput").ap()
    W2_d = nc.dram_tensor("W2", [D, D], f32, kind="ExternalInput").ap()
    b2_d = nc.dram_tensor("b2", [D, 1], f32, kind="ExternalInput").ap()
    Wl1_d = nc.dram_tensor("Wl1", [256, D], f32, kind="ExternalInput").ap()
    bl1_d = nc.dram_tensor("bl1", [256, 1], f32, kind="ExternalInput").ap()
    Wl2_d = nc.dram_tensor("Wl2", [256, 256], f32, kind="ExternalInput").ap()
    bl2_d = nc.dram_tensor("bl2", [256, 1], f32, kind="ExternalInput").ap()
    Wl3_d = nc.dram_tensor("Wl3", [OUT, 256], f32, kind="ExternalInput").ap()
    bl3_d = nc.dram_tensor("bl3", [OUT, 1], f32, kind="ExternalInput").ap()
    y_d = nc.dram_tensor("y", [BS, OUT], f32, kind="ExternalOutput").ap()

    with tile.TileContext(nc) as tc:
        with (
            tc.tile_pool(name="const", bufs=1) as const,
            tc.tile_pool(name="wtmp", bufs=2) as wtmp,
            tc.tile_pool(name="xnat", bufs=bufs[0]) as xnat_pool,
            tc.tile_pool(name="xt", bufs=bufs[1]) as xt_pool,
            tc.tile_pool(name="hrelu", bufs=bufs[3]) as hrelu_pool,
            tc.tile_pool(name="smalls", bufs=4) as smalls,
            tc.tile_pool(name="ps_h", bufs=bufs[2], space="PSUM") as ps_h,
            tc.tile_pool(name="ps_misc", bufs=2, space="PSUM") as ps_misc,
        ):
            # ---------------- one-time setup ----------------
            I128 = const.tile([128, 128], f32, tag="I128")
            make_identity(nc, I128[:])

            W1_sb = const.tile([64, 3 * D], f32, tag="W1_sb")
            nc.sync.dma_start(out=W1_sb[:], in_=W1_d[:])
            W2_sb = const.tile([64, 64], f32, tag="W2_sb")
            nc.sync.dma_start(out=W2_sb[:], in_=W2_d[:])

            # W_pack[k] = block-diag(w_k^T, w_k^T) in bf16
            W_pack = []
            for k in range(3):
                wp = const.tile([128, 128], bf16, tag=f"wpack{k}", name=f"wpack{k}")
                nc.vector.memset(wp[:], 0.0)
                pw = ps_misc.tile([128, 64], f32, tag="pm", name="pm1")
                nc.tensor.matmul(
                    out=pw[0:64, :], lhsT=W1_sb[:, k * 64:(k + 1) * 64],
                    rhs=I128[0:64, 0:64], start=True, stop=True)
                nc.tensor.matmul(
                    out=pw[64:128, :], lhsT=W1_sb[:, k * 64:(k + 1) * 64],
                    rhs=I128[0:64, 0:64], start=True, stop=True,
                    tile_position=(0, 64))
                nc.vector.tensor_copy(wp[0:64, 0:64], pw[0:64, :])
                nc.vector.tensor_copy(wp[64:128, 64:128], pw[64:128, :])
                W_pack.append(wp)

            # W2^T (f32, for the post-pool pointwise linear)
            W2T = const.tile([64, 64], f32, tag="W2T")
            pw2 = ps_misc.tile([64, 64], f32, tag="pm", name="pm2")
            nc.tensor.matmul(out=pw2[:], lhsT=W2_sb[:], rhs=I128[0:64, 0:64],
                             start=True, stop=True)
            nc.vector.tensor_copy(W2T[:], pw2[:])

            # selector for the mask matmul: row0 -> partitions 0:64 (even
            # item of the pair), row1 -> partitions 64:128 (odd item)
            # sel2[j, m] = 1 iff floor(m/64) == j, built via two affine
            # selects (engines can't address partition 1 directly)
            sel2 = const.tile([2, 128], bf16, tag="sel2")
            nc.vector.memset(sel2[:], 1.0)
            nc.gpsimd.affine_select(
                out=sel2[:], in_=sel2[:], pattern=[[1, 128]],
                compare_op=OP.is_ge, fill=0.0, base=0, channel_multiplier=-64)
            nc.gpsimd.affine_select(
                out=sel2[:], in_=sel2[:], pattern=[[-1, 128]],
                compare_op=OP.is_ge, fill=0.0, base=63, channel_multiplier=64)

            b_pack = const.tile([128, 1], f32, tag="b_pack")
            nc.sync.dma_start(out=b_pack[0:64, :], in_=b1_d[:])
            nc.sync.dma_start(out=b_pack[64:128, :], in_=b1_d[:])
            b2_sb = const.tile([64, 1], f32, tag="b2_sb")
            nc.sync.dma_start(out=b2_sb[:], in_=b2_d[:])

            # lengths: column [16,1] and row [1,16] copies
            lens_c = smalls.tile([BS, 1], i32, tag="lens_c")
            nc.sync.dma_start(out=lens_c[:], in_=len_d[:])
            lens_r = smalls.tile([1, BS], i32, tag="lens_r")
            nc.sync.dma_start(out=lens_r[:], in_=len_d.rearrange("a b -> b a"))
            lens_m2 = const.tile([BS, 1], f32, tag="lens_m2")
            nc.vector.tensor_scalar(
                out=lens_m2[:], in0=lens_c[:], scalar1=2.0, scalar2=None,
                op0=OP.subtract)
            lens_m2r = smalls.tile([1, BS], f32, tag="lens_m2r")
            nc.vector.tensor_scalar(
                out=lens_m2r[:], in0=lens_r[:], scalar1=2.0, scalar2=None,
                op0=OP.subtract)
            inv_r = smalls.tile([1, BS], f32, tag="inv_r")
            nc.vector.reciprocal(inv_r[:], lens_m2r[:])
            ones1 = const.tile([1, 128], f32, tag="ones1")
            nc.vector.memset(ones1[:], 1.0)
            # broadcast 1/(L-2) of every item to all 128 partitions
            pinv = ps_misc.tile([128, BS], f32, tag="pm", name="pm3")
            nc.tensor.matmul(out=pinv[:], lhsT=ones1[:], rhs=inv_r[:],
                             start=True, stop=True)
            inv_all = const.tile([128, BS], f32, tag="inv_all")
            nc.vector.tensor_copy(inv_all[:], pinv[:])

            # c_all[i, t] = NEG if t >= L_i - 2 else 0   (bf16)
            iota16 = const.tile([BS, T], f32, tag="iota16")
            nc.gpsimd.iota(out=iota16[:], pattern=[[1, T]], base=0,
                           channel_multiplier=0,
                           allow_small_or_imprecise_dtypes=True)
            c_all = const.tile([BS, T], bf16, tag="c_all")
            nc.vector.tensor_scalar(
                out=c_all[:], in0=iota16[:], scalar1=lens_m2[:], scalar2=NEG,
                op0=OP.is_ge, op1=OP.mult)
            # pair layout: row j of c_pair = item 2p+j at free offset p*T
            c_pair = const.tile([2, NPAIR * T], bf16, tag="c_pair")
            for p in range(NPAIR):
                nc.sync.dma_start(
                    out=c_pair[:, p * T:(p + 1) * T],
                    in_=c_all[2 * p:2 * p + 2, :])

            # MLP weights, transposed on PE into [in, out] layout
            Wl1T = const.tile([64, 256], f32, tag="Wl1T")
            for mc in range(2):
                wtile = wtmp.tile([128, 64], f32, tag="wl1_chunk")
                nc.sync.dma_start(out=wtile[:], in_=Wl1_d[mc * 128:(mc + 1) * 128, :])
                pt = ps_misc.tile([64, 128], f32, tag="pm", name="pm4")
                nc.tensor.matmul(out=pt[:], lhsT=wtile[:], rhs=I128[:],
                                 start=True, stop=True)
                nc.vector.tensor_copy(Wl1T[:, mc * 128:(mc + 1) * 128], pt[:])

            Wl2T = [const.tile([128, 256], f32, tag=f"Wl2T{kc}", name=f"Wl2T{kc}") for kc in range(2)]
            for mc in range(2):
                wtile = wtmp.tile([128, 256], f32, tag="wl2_chunk")
                nc.sync.dma_start(out=wtile[:], in_=Wl2_d[mc * 128:(mc + 1) * 128, :])
                for kc in range(2):
                    pt = ps_misc.tile([128, 128], f32, tag="pm", name="pm5")
                    nc.tensor.matmul(out=pt[:], lhsT=wtile[:, kc * 128:(kc + 1) * 128],
                                     rhs=I128[:], start=True, stop=True)
                    nc.vector.tensor_copy(
                        Wl2T[kc][:, mc * 128:(mc + 1) * 128], pt[:])

            Wl3T = [const.tile([128, OUT], f32, tag=f"Wl3T{kc}", name=f"Wl3T{kc}") for kc in range(2)]
            for mc in range(4):
                wtile = wtmp.tile([128, 256], f32, tag="wl3_chunk")
                nc.sync.dma_start(out=wtile[:], in_=Wl3_d[mc * 128:(mc + 1) * 128, :])
                for kc in range(2):
                    pt = ps_misc.tile([128, 128], f32, tag="pm", name="pm6")
                    nc.tensor.matmul(out=pt[:], lhsT=wtile[:, kc * 128:(kc + 1) * 128],
                                     rhs=I128[:], start=True, stop=True)
                    nc.vector.tensor_copy(
                        Wl3T[kc][:, mc * 128:(mc + 1) * 128], pt[:])

            bl1_sb = [const.tile([128, 1], f32, tag=f"bl1_{m}", name=f"bl1_{m}") for m in range(2)]
            bl2_sb = [const.tile([128, 1], f32, tag=f"bl2_{m}", name=f"bl2_{m}") for m in range(2)]
            bl3_sb = [const.tile([128, 1], f32, tag=f"bl3_{m}", name=f"bl3_{m}") for m in range(4)]
            for m in range(2):
                nc.sync.dma_start(out=bl1_sb[m][:], in_=bl1_d[m * 128:(m + 1) * 128, :])
                nc.sync.dma_start(out=bl2_sb[m][:], in_=bl2_d[m * 128:(m + 1) * 128, :])
            for m in range(4):
                nc.sync.dma_start(out=bl3_sb[m][:], in_=bl3_d[m * 128:(m + 1) * 128, :])

            pooled_all = const.tile([128, NPAIR], f32, tag="pooled_all")

            # x viewed so frame index splits as 2048*h + 128*c + t
            x_v = x_d.rearrange("b (h c t d) -> b h t c d", h=2, c=16, t=128, d=D)

            # ---------------- per-pair streaming loop ----------------
            import contextlib
            for rep in range(repeat):
              with (tc.For_i(0, loop_n, 1) if loop_n else
                    contextlib.nullcontext()):
                for p in range(NPAIR):
                  xnat = xnat_pool.tile([128, 32, 128], bf16)
                  for h in range(2):  # T halves
                      for it in range(2):  # item within pair
                          nc.gpsimd.dma_start(
                              out=xnat[:, 16 * h:16 * h + 16, 64 * it:64 * it + 64],
                              in_=x_v[2 * p + it, h])
                  if stage < 2:
                      continue
                  xt = xt_pool.tile([128, T], bf16, name="xt", tag="xt")
                  # batched xbar transpose: out[d, c, t] = in[t, c, d]
                  nc.sync.dma_start(
                      out=xt[:].rearrange("p (c t) -> p c t", c=32),
                      in_=xnat[:], transpose=True)
                  if stage < 3:
                      continue

                  partials = (smalls.tile([128, NCHUNK], f32, tag="partials",
                                          name="partials")
                              if stage >= 4 else None)
                  for n in range(NCHUNK):
                      N = TC if n < NCHUNK - 1 else TC - 2
                      psum = ps_h.tile([128, TC], f32)
                      for k in range(3):
                          nc.tensor.matmul(
                              out=psum[:, :N], lhsT=W_pack[k][:],
                              rhs=xt[:, TC * n + k:TC * n + k + N],
                              start=(k == 0), stop=False)
                      nc.tensor.matmul(
                          out=psum[:, :N], lhsT=sel2[:],
                          rhs=c_pair[:, T * p + TC * n:T * p + TC * n + N],
                          start=False, stop=True)
                      if stage < 4:
                          continue
                      hrelu = hrelu_pool.tile([128, TC], bf16)
                      nc.scalar.activation(
                          out=hrelu[:, :N], in_=psum[:, :N], func=AF.Relu,
                          bias=b_pack[:], accum_out=partials[:, n:n + 1])

                  if stage < 5:
                      continue
                  pool_sum = smalls.tile([128, 1], f32, tag="pool_sum")
                  nc.vector.tensor_reduce(out=pool_sum[:], in_=partials[:],
                                          axis=AX.X, op=OP.add)
                  inv_pack = smalls.tile([128, 1], f32, tag="inv_pack")
                  nc.vector.tensor_copy(inv_pack[0:64, :],
                                        inv_all[0:64, 2 * p:2 * p + 1])
                  nc.vector.tensor_copy(inv_pack[64:128, :],
                                        inv_all[64:128, 2 * p + 1:2 * p + 2])
                  nc.vector.tensor_scalar(
                      out=pooled_all[:, p:p + 1], in0=pool_sum[:],
                      scalar1=inv_pack[:], scalar2=None, op0=OP.mult)

              if stage < 6:
                  nc.sync.dma_start(out=y_d[:, 0:NPAIR], in_=pooled_all[0:BS, :])
              if stage < 5:
                  nc.vector.memset(pooled_all[:], 0.0)
              # ---------------- pooled -> td2 -> MLP (f32) ----------------
              # PL2 cols: [item0,2,..,14, item1,3,..,15]
              PL2 = const.tile([64, BS], f32, tag="PL2")
              nc.vector.tensor_copy(PL2[:, 0:NPAIR], pooled_all[0:64, :])
              nc.sync.dma_start(out=PL2[:, NPAIR:BS], in_=pooled_all[64:128, :])

              # td2: p2 = W2 @ pooled + b2
              pp2 = ps_misc.tile([64, BS], f32, tag="pm", name="pm7")
              nc.tensor.matmul(out=pp2[:], lhsT=W2T[:], rhs=PL2[:],
                               start=True, stop=True)
              PL3 = const.tile([64, BS], f32, tag="PL3")
              nc.scalar.activation(out=PL3[:], in_=pp2[:], func=AF.Identity,
                                   bias=b2_sb[:])

              z1 = [const.tile([128, BS], f32, tag=f"z1_{m}", name=f"z1_{m}") for m in range(2)]
              for m in range(2):
                  pz = ps_misc.tile([128, BS], f32, tag="pm", name="pm8")
                  nc.tensor.matmul(out=pz[:], lhsT=Wl1T[:, m * 128:(m + 1) * 128],
                                   rhs=PL3[:], start=True, stop=True)
                  nc.scalar.activation(out=z1[m][:], in_=pz[:], func=AF.Relu,
                                       bias=bl1_sb[m][:])
              z2 = [const.tile([128, BS], f32, tag=f"z2_{m}", name=f"z2_{m}") for m in range(2)]
              for m in range(2):
                  pz = ps_misc.tile([128, BS], f32, tag="pm", name="pm9")
                  for kc in range(2):
                      nc.tensor.matmul(out=pz[:], lhsT=Wl2T[kc][:, m * 128:(m + 1) * 128],
                                       rhs=z1[kc][:], start=(kc == 0), stop=(kc == 1))
                  nc.scalar.activation(out=z2[m][:], in_=pz[:], func=AF.Relu,
                                       bias=bl2_sb[m][:])
              y_sb = const.tile([BS, OUT], f32, tag="y_sb")
              for m in range(4):
                  pz = ps_misc.tile([128, BS], f32, tag="pm", name="pm10")
                  for kc in range(2):
                      nc.tensor.matmul(out=pz[:], lhsT=Wl3T[kc][:, m * 128:(m + 1) * 128],
                                       rhs=z2[kc][:], start=(kc == 0), stop=(kc == 1))
                  ym = const.tile([128, BS], f32, tag=f"ym_{m}", name=f"ym_{m}")
                  nc.scalar.activation(out=ym[:], in_=pz[:], func=AF.Identity,
                                       bias=bl3_sb[m][:])
                  # transpose [feat, item] -> [item, feat]
                  pt = ps_misc.tile([BS, 128], f32, tag="pm", name="pm11")
                  nc.tensor.matmul(out=pt[:], lhsT=ym[:], rhs=I128[:],
                                   start=True, stop=True)
                  nc.vector.tensor_copy(y_sb[:, m * 128:(m + 1) * 128], pt[:])

              y_v = y_d.rearrange("(j two) f -> two j f", two=2)
              nc.sync.dma_start(out=y_v[0], in_=y_sb[0:NPAIR, :])
              nc.sync.dma_start(out=y_v[1], in_=y_sb[NPAIR:BS, :])

    nc.compile()
    return nc


def _get_nc():
    if "nc" not in _CACHE:
        _CACHE["nc"] = _build()
    return _CACHE["nc"]


def kernel(x, lengths, W1, b1, W2, b2, Wl1, bl1, Wl2, bl2, Wl3, bl3,
           _want_trace=False, **_ignored):
    from concourse.bass_utils import run_bass_kernel_spmd

    x = np.asarray(x, np.float32)
    lengths = np.asarray(lengths, np.int32)
    shared = {
        "W1": np.asarray(W1, np.float32)[:, 0].reshape(D, 3 * D).copy(),
        "b1": np.asarray(b1, np.float32).reshape(D, 1).copy(),
        "W2": np.asarray(W2, np.float32).reshape(D, D).copy(),
        "b2": np.asarray(b2, np.float32).reshape(D, 1).copy(),
        "Wl1": np.asarray(Wl1, np.float32),
        "bl1": np.asarray(bl1, np.float32).reshape(256, 1).copy(),
        "Wl2": np.asarray(Wl2, np.float32),
        "bl2": np.asarray(bl2, np.float32).reshape(256, 1).copy(),
        "Wl3": np.asarray(Wl3, np.float32),
        "bl3": np.asarray(bl3, np.float32).reshape(OUT, 1).copy(),
    }
    in_maps = []
    for c in range(NCORES):
        sl = slice(c * BS, (c + 1) * BS)
        in_maps.append({
            "x": np.ascontiguousarray(x[sl].reshape(BS, T * D)),
            "lengths": np.ascontiguousarray(lengths[sl].reshape(BS, 1)),
            **shared,
        })
    nc = _get_nc()
    res = run_bass_kernel_spmd(nc, in_maps, list(range(NCORES)),
                               trace=_want_trace)
    out = np.concatenate([res.results[c]["y"] for c in range(NCORES)], axis=0)
    if _want_trace:
        _CACHE["last_result"] = res
    return out



# revision 10
# speedup vs baseline: 3.1919x; 3.1919x over previous
"""Trainium2 Bass kernel for ConvolutionFeatureProcessor.

Math (per item, matching the jax reference):
  h[t]   = relu(b1 + sum_k x[t+k] @ w1k^T)          t in [0, T-2)
  pooled = (1/(L-2)) * sum_{t<L-2} h[t]             (masked mean)
  p2     = W2 @ pooled + b2      (td2 linear -> commutes with the mean)
  out    = MLP(p2)               (64 -> 256 -> 256 -> 512)

Strategy (8 cores, data parallel over the batch, ragged-aware):
  - Host sorts items by length and packs 2 similar-length items per
    128-partition "slot"; slot s processes only ceil(Lmax_s/512) chunks
    of 512 frames (compile-time constants -> program cache key).
  - Host pre-transposes x to [d, t] layout, zero-pads each item's tail,
    and casts to bf16, so the device does a single contiguous bf16 load
    per slot (half the HBM traffic, no on-device transpose).
  - No masking on device: tails are zero-padded, and the host computes
    the exact correction (2 boundary frames + count * relu(b1)) that the
    device subtracts from the pooled sum.
  - Conv: weight-stationary k-sweeps (3 matmuls per chunk, block-diag
    w_k^T lhsT), one PSUM bank per chunk (up to 8 in flight).
  - relu+bias+pool-accum in ONE tensor_scalar op per chunk, alternating
    between the Vector and GpSimd engines.
  - td2 is folded into the first MLP layer on host; MLP runs in bf16
    feature-major; output is written [feat, item] and untangled on host.
"""

import numpy as np
import ml_dtypes

B, T, D, OUT = 128, 4096, 64, 512
NCORES = 8
BS = B // NCORES       # items per core
NSLOT = BS // 2        # pair-slots per core
TC = 512               # conv chunk (free dim per matmul / psum bank)

_CACHE = {}


def _build(ns_list, dbg=False):
    """Compile the SPMD program for per-slot chunk counts `ns_list`."""
    import concourse.bacc as bacc
    import concourse.mybir as mybir
    import concourse.tile as tile

    f32 = mybir.dt.float32
    bf16 = mybir.dt.bfloat16
    AX = mybir.AxisListType
    OP = mybir.AluOpType
    AF = mybir.ActivationFunctionType

    F_tot = sum(TC * n for n in ns_list)

    nc = bacc.Bacc("TRN2", target_bir_lowering=False, debug=False)

    xt_d = nc.dram_tensor("xt", [128, F_tot], bf16, kind="ExternalInput").ap()
    wpk_d = nc.dram_tensor("wpk", [128, 3 * 128], bf16, kind="ExternalInput").ap()
    bpk_d = nc.dram_tensor("bpk", [128, 1], f32, kind="ExternalInput").ap()
    inv_d = nc.dram_tensor("inv", [128, NSLOT], f32, kind="ExternalInput").ap()
    corr_d = nc.dram_tensor("corr", [64, BS], f32, kind="ExternalInput").ap()
    wl1_d = nc.dram_tensor("wl1", [64, 256], bf16, kind="ExternalInput").ap()
    wl2_d = nc.dram_tensor("wl2", [128, 512], bf16, kind="ExternalInput").ap()
    wl3_d = nc.dram_tensor("wl3", [128, 1024], bf16, kind="ExternalInput").ap()
    bl1_d = nc.dram_tensor("bl1", [128, 2], f32, kind="ExternalInput").ap()
    bl2_d = nc.dram_tensor("bl2", [128, 2], f32, kind="ExternalInput").ap()
    bl3_d = nc.dram_tensor("bl3", [128, 4], f32, kind="ExternalInput").ap()
    y_d = nc.dram_tensor("y", [128, 4 * BS], f32, kind="ExternalOutput").ap()
    if dbg:
        dbg_pool = nc.dram_tensor("dbg_pool", [128, NSLOT], f32,
                                  kind="ExternalOutput").ap()
        dbg_part = nc.dram_tensor("dbg_part", [128, 8], f32,
                                  kind="ExternalOutput").ap()
        dbg_h = nc.dram_tensor("dbg_h", [128, TC], f32,
                               kind="ExternalOutput").ap()
        dbg_pl = nc.dram_tensor("dbg_pl", [64, BS], f32,
                                kind="ExternalOutput").ap()

    with tile.TileContext(nc) as tc:
        with (
            tc.tile_pool(name="const", bufs=1) as const,
            tc.tile_pool(name="xt", bufs=2) as xt_pool,
            tc.tile_pool(name="junk", bufs=1) as junk_pool,
            tc.tile_pool(name="smalls", bufs=2) as smalls,
            tc.tile_pool(name="ps", bufs=8, space="PSUM") as ps,
        ):
            # ---------------- constant loads (pure DMA) ----------------
            W_pack = const.tile([128, 3 * 128], bf16, tag="wpk")
            nc.sync.dma_start(out=W_pack[:], in_=wpk_d[:])
            b_pack = const.tile([128, 1], f32, tag="bpk")
            nc.sync.dma_start(out=b_pack[:], in_=bpk_d[:])
            inv_all = const.tile([128, NSLOT], f32, tag="inv")
            nc.sync.dma_start(out=inv_all[:], in_=inv_d[:])
            corr_sb = const.tile([64, BS], f32, tag="corr")
            nc.sync.dma_start(out=corr_sb[:], in_=corr_d[:])
            Wl1T = const.tile([64, 256], bf16, tag="wl1")
            nc.scalar.dma_start(out=Wl1T[:], in_=wl1_d[:])
            Wl2T = const.tile([128, 512], bf16, tag="wl2")
            nc.scalar.dma_start(out=Wl2T[:], in_=wl2_d[:])
            Wl3T = const.tile([128, 1024], bf16, tag="wl3")
            nc.scalar.dma_start(out=Wl3T[:], in_=wl3_d[:])
            bl1_sb = const.tile([128, 2], f32, tag="bl1")
            nc.scalar.dma_start(out=bl1_sb[:], in_=bl1_d[:])
            bl2_sb = const.tile([128, 2], f32, tag="bl2")
            nc.scalar.dma_start(out=bl2_sb[:], in_=bl2_d[:])
            bl3_sb = const.tile([128, 4], f32, tag="bl3")
            nc.scalar.dma_start(out=bl3_sb[:], in_=bl3_d[:])

            pooled_all = const.tile([128, NSLOT], f32, tag="pooled_all")
            junk_v = junk_pool.tile([128, TC], bf16, tag="junk_v")
            junk_g = junk_pool.tile([128, TC], bf16, tag="junk_g")
            zeros_sb = const.tile([128, TC], bf16, tag="zeros_sb")
            nc.vector.memset(zeros_sb[:], 0.0)

            # ---------------- per-slot streaming conv ----------------
            off = 0
            for s in range(NSLOT):
                n_s = ns_list[s]
                F_s = TC * n_s
                xt = xt_pool.tile([128, F_s], bf16, name=f"xt{s}", tag="xt")
                nc.sync.dma_start(out=xt[:], in_=xt_d[:, off:off + F_s])
                off += F_s

                psums = [ps.tile([128, TC], f32, name=f"ps{s}_{n}", tag="ps")
                         for n in range(n_s)]
                for k in range(3):
                    lhsT = W_pack[:, 128 * k:128 * (k + 1)]
                    for n in range(n_s):
                        N = TC if n < n_s - 1 else TC - 2
                        nc.tensor.matmul(
                            out=psums[n][:, :N], lhsT=lhsT,
                            rhs=xt[:, TC * n + k:TC * n + k + N],
                            start=(k == 0), stop=(k == 2))

                partials = smalls.tile([128, n_s], f32, name=f"pt{s}",
                                       tag="partials")
                for n in range(n_s):
                    N = TC if n < n_s - 1 else TC - 2
                    if n % 3 == 2:
                        # scalar engine: out/accum = relu(in + bias), sum
                        nc.scalar.activation(
                            out=junk_g[:, :N], in_=psums[n][:, :N],
                            func=AF.Relu, bias=b_pack[:],
                            accum_out=partials[:, n:n + 1])
                    else:
                        # DVE: out = (in + bias) max 0; accum_out = sum(out)
                        nc.vector.scalar_tensor_tensor(
                            out=junk_v[:, :N], in0=psums[n][:, :N],
                            scalar=b_pack[:], in1=zeros_sb[:, :N],
                            op0=OP.add, op1=OP.max,
                            accum_out=partials[:, n:n + 1])
                    if dbg and s == 0 and n == 0:
                        hcp = smalls.tile([128, TC], f32, tag="hcp")
                        nc.vector.tensor_copy(hcp[:, :N], junk_v[:, :N])
                        nc.sync.dma_start(out=dbg_h[:, :N], in_=hcp[:, :N])

                pool_sum = smalls.tile([128, 1], f32, name=f"psum{s}",
                                       tag="pool_sum")
                if n_s > 1:
                    nc.vector.tensor_reduce(out=pool_sum[:], in_=partials[:],
                                            axis=AX.X, op=OP.add)
                else:
                    nc.vector.tensor_copy(pool_sum[:], partials[:])
                nc.vector.tensor_scalar(
                    out=pooled_all[:, s:s + 1], in0=pool_sum[:],
                    scalar1=inv_all[:, s:s + 1], scalar2=None, op0=OP.mult)
                if dbg and s == 0:
                    nc.sync.dma_start(out=dbg_part[:, :n_s], in_=partials[:])

            # ---------------- pooled -> MLP (bf16) ----------------
            # PL cols: item j = 2s+pos; even cols from partitions 0:64,
            # odd cols from partitions 64:128 (partition move via DMA).
            PLf = const.tile([64, BS], f32, tag="PLf")
            pv = pooled_all[:].rearrange("p (s one) -> p s one", one=1)
            plv = PLf[:].rearrange("p (s two) -> p s two", two=2)
            nc.vector.tensor_copy(plv[:, :, 0:1], pv[0:64])
            nc.sync.dma_start(out=plv[:, :, 1:2], in_=pv[64:128])
            # subtract host correction; cast to bf16
            PL = const.tile([64, BS], bf16, tag="PL")
            nc.vector.tensor_tensor(out=PL[:], in0=PLf[:], in1=corr_sb[:],
                                    op=OP.subtract)
            if dbg:
                nc.sync.dma_start(out=dbg_pool[:], in_=pooled_all[:])
                PLc = const.tile([64, BS], f32, tag="PLc")
                nc.vector.tensor_copy(PLc[:], PL[:])
                nc.sync.dma_start(out=dbg_pl[:], in_=PLc[:])

            z1 = const.tile([128, 2 * BS], bf16, tag="z1")
            for m in range(2):
                pz = ps.tile([128, TC], f32, name=f"pz1_{m}", tag="ps")[:, :BS]
                nc.tensor.matmul(out=pz[:], lhsT=Wl1T[:, m * 128:(m + 1) * 128],
                                 rhs=PL[:], start=True, stop=True)
                nc.scalar.activation(out=z1[:, m * BS:(m + 1) * BS], in_=pz[:],
                                     func=AF.Relu, bias=bl1_sb[:, m:m + 1])
            z2 = const.tile([128, 2 * BS], bf16, tag="z2")
            for m in range(2):
                pz = ps.tile([128, TC], f32, name=f"pz2_{m}", tag="ps")[:, :BS]
                for kc in range(2):
                    nc.tensor.matmul(
                        out=pz[:],
                        lhsT=Wl2T[:, 256 * kc + 128 * m:256 * kc + 128 * (m + 1)],
                        rhs=z1[:, kc * BS:(kc + 1) * BS],
                        start=(kc == 0), stop=(kc == 1))
                nc.scalar.activation(out=z2[:, m * BS:(m + 1) * BS], in_=pz[:],
                                     func=AF.Relu, bias=bl2_sb[:, m:m + 1])
            y_sb = const.tile([128, 4 * BS], f32, tag="y_sb")
            for m in range(4):
                pz = ps.tile([128, TC], f32, name=f"pz3_{m}", tag="ps")[:, :BS]
                for kc in range(2):
                    nc.tensor.matmul(
                        out=pz[:],
                        lhsT=Wl3T[:, 512 * kc + 128 * m:512 * kc + 128 * (m + 1)],
                        rhs=z2[:, kc * BS:(kc + 1) * BS],
                        start=(kc == 0), stop=(kc == 1))
                nc.scalar.activation(out=y_sb[:, m * BS:(m + 1) * BS], in_=pz[:],
                                     func=AF.Identity, bias=bl3_sb[:, m:m + 1])
            nc.sync.dma_start(out=y_d[:], in_=y_sb[:])

    nc.compile()
    return nc


def _get_nc(ns_key):
    nc = _CACHE.get(ns_key)
    if nc is None:
        nc = _CACHE[ns_key] = _build(list(ns_key))
    return nc


def _host_prep(x, lengths, W1, b1, W2, b2, Wl1, bl1, Wl2, bl2, Wl3, bl3):
    bf16 = ml_dtypes.bfloat16
    x = np.asarray(x, np.float32)
    lengths = np.asarray(lengths, np.int32)
    W1 = np.asarray(W1, np.float32)      # [D,1,3,D]
    b1 = np.asarray(b1, np.float32)
    W2 = np.asarray(W2, np.float32).reshape(D, D)
    b2 = np.asarray(b2, np.float32)
    Wl1 = np.asarray(Wl1, np.float32)    # [256, D]
    bl1 = np.asarray(bl1, np.float32)
    Wl2 = np.asarray(Wl2, np.float32)
    bl2 = np.asarray(bl2, np.float32)
    Wl3 = np.asarray(Wl3, np.float32)
    bl3 = np.asarray(bl3, np.float32)

    # ---- sort by length, assign ranks: slot s <- ranks [16s, 16s+16),
    # core c gets ranks 16s+2c (pos 0 -> partitions 0:64) and 16s+2c+1.
    order = np.argsort(-lengths, kind="stable")
    Ls = lengths[order]
    ns_list = []
    for s in range(NSLOT):
        mx = int(Ls[16 * s:16 * s + 16].max())
        ns_list.append(max(1, -(-mx // TC)))
    ns_key = tuple(ns_list)
    F_tot = sum(TC * n for n in ns_list)

    wk = W1[:, 0]                        # [F, 3, D]; h += x[t+k] @ wk[:,k].T

    # ---- per-item device frames + exact host-side pooling correction
    # S_dev(item) = sum_{t<512n_s-2} relu(h~[t]) with x zero-padded at L.
    # corr = S_dev - S_true, pre-divided by (L-2).
    relu_b1 = np.maximum(b1, 0.0)        # [D]
    corr_sorted = np.zeros((B, D), np.float32)
    for r in range(B):
        it = order[r]
        L = int(lengths[it])
        M = TC * ns_list[r // 16]        # frames loaded for this item
        c = np.zeros(D, np.float32)
        c += max(0, M - 2 - L) * relu_b1
        if L - 2 <= M - 3:
            c += np.maximum(b1 + wk[:, 0] @ x[it, L - 2] + wk[:, 1] @ x[it, L - 1], 0.0)
        if L - 1 <= M - 3:
            c += np.maximum(b1 + wk[:, 0] @ x[it, L - 1], 0.0)
        corr_sorted[r] = c / (L - 2)

    # ---- shared (weight) inputs, host-transposed/packed
    wpk = np.zeros((128, 3 * 128), np.float32)
    for k in range(3):
        wkT = wk[:, k].T                 # [D(in), F(out)]
        wpk[0:64, 128 * k:128 * k + 64] = wkT
        wpk[64:128, 128 * k + 64:128 * (k + 1)] = wkT
    bpk = np.concatenate([b1, b1]).reshape(128, 1)

    # fold td2 into layer 1:  z1 = relu(Wl1 @ (W2 p + b2) + bl1)
    Wl1f = Wl1 @ W2                      # [256, 64]
    bl1f = Wl1 @ b2 + bl1                # [256]
    wl1 = np.ascontiguousarray(Wl1f.T)   # [64, 256]
    wl2 = np.ascontiguousarray(Wl2.T)    # [256, 256] -> [128, 2*256]
    wl2 = wl2.reshape(2, 128, 256).transpose(1, 0, 2).reshape(128, 512)
    wl3 = np.ascontiguousarray(Wl3.T)    # [256, 512] -> [128, 2*512]
    wl3 = wl3.reshape(2, 128, 512).transpose(1, 0, 2).reshape(128, 1024)
    shared = {
        "wpk": wpk.astype(bf16),
        "bpk": bpk,
        "wl1": wl1.astype(bf16),
        "wl2": np.ascontiguousarray(wl2).astype(bf16),
        "wl3": np.ascontiguousarray(wl3).astype(bf16),
        "bl1": np.ascontiguousarray(bl1f.reshape(2, 128).T),
        "bl2": np.ascontiguousarray(bl2.reshape(2, 128).T),
        "bl3": np.ascontiguousarray(bl3.reshape(4, 128).T),
    }

    # ---- per-core ragged transposed bf16 x, inv, corr
    in_maps = []
    for c in range(NCORES):
        xt = np.zeros((128, F_tot), bf16)
        inv = np.zeros((128, NSLOT), np.float32)
        corr = np.zeros((64, BS), np.float32)
        off = 0
        for s in range(NSLOT):
            F_s = TC * ns_list[s]
            for pos in range(2):
                r = 16 * s + 2 * c + pos
                it = order[r]
                L = int(lengths[it])
                n_use = min(L, F_s)
                xt[64 * pos:64 * pos + 64, off:off + n_use] = \
                    x[it, :n_use].T.astype(bf16)
                inv[64 * pos:64 * pos + 64, s] = 1.0 / (L - 2)
                corr[:, 2 * s + pos] = corr_sorted[r]
            off += F_s
        in_maps.append({"xt": xt, "inv": inv, "corr": corr, **shared})

    return ns_key, in_maps, order


def _gather_out(per_core_y, order):
    # y_sb[:, m*BS + j] = feats[m*128:(m+1)*128] of device item j = 2s+pos
    # on core c  -> global rank 16s+2c+pos.
    out = np.empty((B, OUT), np.float32)
    for c in range(NCORES):
        Y = np.asarray(per_core_y[c], np.float32)   # [128, 4*BS]
        feats = Y.reshape(128, 4, BS).transpose(2, 1, 0).reshape(BS, OUT)
        for j in range(BS):
            s, pos = divmod(j, 2)
            out[order[16 * s + 2 * c + pos]] = feats[j]
    return out


def kernel(x, lengths, W1, b1, W2, b2, Wl1, bl1, Wl2, bl2, Wl3, bl3,
           _want_trace=False, **_ignored):
    from concourse.bass_utils import run_bass_kernel_spmd

    ns_key, in_maps, order = _host_prep(
        x, lengths, W1, b1, W2, b2, Wl1, bl1, Wl2, bl2, Wl3, bl3)
    nc = _get_nc(ns_key)
    res = run_bass_kernel_spmd(nc, in_maps, list(range(NCORES)),
                               trace=_want_trace)
    if _want_trace:
        _CACHE["last_result"] = res
    return _gather_out([res.results[c]["y"] for c in range(NCORES)], order)


# revision 11
# speedup vs baseline: 3.1984x; 1.0020x over previous
"""Trainium2 Bass kernel for ConvolutionFeatureProcessor.

Math (per item, matching the jax reference):
  h[t]   = relu(b1 + sum_k x[t+k] @ w1k^T)          t in [0, T-2)
  pooled = (1/(L-2)) * sum_{t<L-2} h[t]             (masked mean)
  p2     = W2 @ pooled + b2      (td2 linear -> commutes with the mean)
  out    = MLP(p2)               (64 -> 256 -> 256 -> 512)

Strategy (8 cores, data parallel over the batch, ragged-aware):
  - Host sorts items by length and packs 2 similar-length items per
    128-partition "slot"; slot s processes only ceil(Lmax_s/512) chunks
    of 512 frames (compile-time constants -> program cache key).
  - Host pre-transposes x to [d, t] layout, zero-pads each item's tail,
    and casts to bf16, so the device does a single contiguous bf16 load
    per slot (half the HBM traffic, no on-device transpose).
  - No masking on device: tails are zero-padded, and the host computes
    the exact correction (2 boundary frames + count * relu(b1)) that the
    device subtracts from the pooled sum.
  - Conv: weight-stationary k-sweeps (3 matmuls per chunk, block-diag
    w_k^T lhsT), one PSUM bank per chunk (up to 8 in flight).
  - relu+bias+pool-accum in ONE tensor_scalar op per chunk, alternating
    between the Vector and GpSimd engines.
  - td2 is folded into the first MLP layer on host; MLP runs in bf16
    feature-major; output is written [feat, item] and untangled on host.
"""

import numpy as np
import ml_dtypes

B, T, D, OUT = 128, 4096, 64, 512
NCORES = 8
BS = B // NCORES       # items per core
NSLOT = BS // 2        # pair-slots per core
TC = 512               # conv chunk (free dim per matmul / psum bank)

_CACHE = {}


def _build(ns_list, dbg=False):
    """Compile the SPMD program for per-slot chunk counts `ns_list`."""
    import concourse.bacc as bacc
    import concourse.mybir as mybir
    import concourse.tile as tile

    f32 = mybir.dt.float32
    bf16 = mybir.dt.bfloat16
    AX = mybir.AxisListType
    OP = mybir.AluOpType
    AF = mybir.ActivationFunctionType

    F_tot = sum(TC * n for n in ns_list)

    nc = bacc.Bacc("TRN2", target_bir_lowering=False, debug=False)

    xts_d = [nc.dram_tensor(f"xt{s}", [128, TC * ns_list[s]], bf16,
                             kind="ExternalInput").ap()
             for s in range(NSLOT)]
    wpk_d = nc.dram_tensor("wpk", [128, 3 * 128], bf16, kind="ExternalInput").ap()
    bpk_d = nc.dram_tensor("bpk", [128, 1], f32, kind="ExternalInput").ap()
    inv_d = nc.dram_tensor("inv", [128, NSLOT], f32, kind="ExternalInput").ap()
    corr_d = nc.dram_tensor("corr", [64, BS], f32, kind="ExternalInput").ap()
    wl1_d = nc.dram_tensor("wl1", [64, 256], bf16, kind="ExternalInput").ap()
    wl2_d = nc.dram_tensor("wl2", [128, 512], bf16, kind="ExternalInput").ap()
    wl3_d = nc.dram_tensor("wl3", [128, 1024], bf16, kind="ExternalInput").ap()
    bl1_d = nc.dram_tensor("bl1", [128, 2], f32, kind="ExternalInput").ap()
    bl2_d = nc.dram_tensor("bl2", [128, 2], f32, kind="ExternalInput").ap()
    bl3_d = nc.dram_tensor("bl3", [128, 4], f32, kind="ExternalInput").ap()
    y_d = nc.dram_tensor("y", [128, 4 * BS], f32, kind="ExternalOutput").ap()
    if dbg:
        dbg_pool = nc.dram_tensor("dbg_pool", [128, NSLOT], f32,
                                  kind="ExternalOutput").ap()
        dbg_part = nc.dram_tensor("dbg_part", [128, 8], f32,
                                  kind="ExternalOutput").ap()
        dbg_h = nc.dram_tensor("dbg_h", [128, TC], f32,
                               kind="ExternalOutput").ap()
        dbg_pl = nc.dram_tensor("dbg_pl", [64, BS], f32,
                                kind="ExternalOutput").ap()

    with tile.TileContext(nc) as tc:
        with (
            tc.tile_pool(name="const", bufs=1) as const,
            tc.tile_pool(name="xt", bufs=3) as xt_pool,
            tc.tile_pool(name="junk", bufs=1) as junk_pool,
            tc.tile_pool(name="smalls", bufs=2) as smalls,
            tc.tile_pool(name="ps", bufs=8, space="PSUM") as ps,
        ):
            # ---------------- constant loads (pure DMA) ----------------
            W_pack = const.tile([128, 3 * 128], bf16, tag="wpk")
            nc.sync.dma_start(out=W_pack[:], in_=wpk_d[:])
            b_pack = const.tile([128, 1], f32, tag="bpk")
            nc.sync.dma_start(out=b_pack[:], in_=bpk_d[:])
            inv_all = const.tile([128, NSLOT], f32, tag="inv")
            nc.sync.dma_start(out=inv_all[:], in_=inv_d[:])
            corr_sb = const.tile([64, BS], f32, tag="corr")
            nc.sync.dma_start(out=corr_sb[:], in_=corr_d[:])
            Wl1T = const.tile([64, 256], bf16, tag="wl1")
            nc.scalar.dma_start(out=Wl1T[:], in_=wl1_d[:])
            Wl2T = const.tile([128, 512], bf16, tag="wl2")
            nc.scalar.dma_start(out=Wl2T[:], in_=wl2_d[:])
            Wl3T = const.tile([128, 1024], bf16, tag="wl3")
            nc.scalar.dma_start(out=Wl3T[:], in_=wl3_d[:])
            bl1_sb = const.tile([128, 2], f32, tag="bl1")
            nc.scalar.dma_start(out=bl1_sb[:], in_=bl1_d[:])
            bl2_sb = const.tile([128, 2], f32, tag="bl2")
            nc.scalar.dma_start(out=bl2_sb[:], in_=bl2_d[:])
            bl3_sb = const.tile([128, 4], f32, tag="bl3")
            nc.scalar.dma_start(out=bl3_sb[:], in_=bl3_d[:])

            pooled_all = const.tile([128, NSLOT], f32, tag="pooled_all")
            junk_v = junk_pool.tile([128, TC], bf16, tag="junk_v")
            junk_g = junk_pool.tile([128, TC], bf16, tag="junk_g")
            zeros_sb = const.tile([128, TC], bf16, tag="zeros_sb")
            nc.vector.memset(zeros_sb[:], 0.0)

            # ---------------- per-slot streaming conv ----------------
            for s in range(NSLOT):
                n_s = ns_list[s]
                F_s = TC * n_s
                xt = xt_pool.tile([128, F_s], bf16, name=f"xt{s}", tag="xt")
                nc.sync.dma_start(out=xt[:], in_=xts_d[s][:])

                psums = [ps.tile([128, TC], f32, name=f"ps{s}_{n}", tag="ps")
                         for n in range(n_s)]
                for k in range(3):
                    lhsT = W_pack[:, 128 * k:128 * (k + 1)]
                    for n in range(n_s):
                        N = TC if n < n_s - 1 else TC - 2
                        nc.tensor.matmul(
                            out=psums[n][:, :N], lhsT=lhsT,
                            rhs=xt[:, TC * n + k:TC * n + k + N],
                            start=(k == 0), stop=(k == 2))

                partials = smalls.tile([128, n_s], f32, name=f"pt{s}",
                                       tag="partials")
                for n in range(n_s):
                    N = TC if n < n_s - 1 else TC - 2
                    if n % 8 in (2, 5, 7):
                        # scalar engine: out/accum = relu(in + bias), sum
                        nc.scalar.activation(
                            out=junk_g[:, :N], in_=psums[n][:, :N],
                            func=AF.Relu, bias=b_pack[:],
                            accum_out=partials[:, n:n + 1])
                    else:
                        # DVE: out = (in + bias) max 0; accum_out = sum(out)
                        nc.vector.scalar_tensor_tensor(
                            out=junk_v[:, :N], in0=psums[n][:, :N],
                            scalar=b_pack[:], in1=zeros_sb[:, :N],
                            op0=OP.add, op1=OP.max,
                            accum_out=partials[:, n:n + 1])
                    if dbg and s == 0 and n == 0:
                        hcp = smalls.tile([128, TC], f32, tag="hcp")
                        nc.vector.tensor_copy(hcp[:, :N], junk_v[:, :N])
                        nc.sync.dma_start(out=dbg_h[:, :N], in_=hcp[:, :N])

                pool_sum = smalls.tile([128, 1], f32, name=f"psum{s}",
                                       tag="pool_sum")
                if n_s > 1:
                    nc.vector.tensor_reduce(out=pool_sum[:], in_=partials[:],
                                            axis=AX.X, op=OP.add)
                else:
                    nc.vector.tensor_copy(pool_sum[:], partials[:])
                nc.vector.tensor_scalar(
                    out=pooled_all[:, s:s + 1], in0=pool_sum[:],
                    scalar1=inv_all[:, s:s + 1], scalar2=None, op0=OP.mult)
                if dbg and s == 0:
                    nc.sync.dma_start(out=dbg_part[:, :n_s], in_=partials[:])

            # ---------------- pooled -> MLP (bf16) ----------------
            # PL cols: item j = 2s+pos; even cols from partitions 0:64,
            # odd cols from partitions 64:128 (partition move via DMA).
            PLf = const.tile([64, BS], f32, tag="PLf")
            pv = pooled_all[:].rearrange("p (s one) -> p s one", one=1)
            plv = PLf[:].rearrange("p (s two) -> p s two", two=2)
            nc.vector.tensor_copy(plv[:, :, 0:1], pv[0:64])
            nc.sync.dma_start(out=plv[:, :, 1:2], in_=pv[64:128])
            # subtract host correction; cast to bf16
            PL = const.tile([64, BS], bf16, tag="PL")
            nc.vector.tensor_tensor(out=PL[:], in0=PLf[:], in1=corr_sb[:],
                                    op=OP.subtract)
            if dbg:
                nc.sync.dma_start(out=dbg_pool[:], in_=pooled_all[:])
                PLc = const.tile([64, BS], f32, tag="PLc")
                nc.vector.tensor_copy(PLc[:], PL[:])
                nc.sync.dma_start(out=dbg_pl[:], in_=PLc[:])

            z1 = const.tile([128, 2 * BS], bf16, tag="z1")
            for m in range(2):
                pz = ps.tile([128, TC], f32, name=f"pz1_{m}", tag="ps")[:, :BS]
                nc.tensor.matmul(out=pz[:], lhsT=Wl1T[:, m * 128:(m + 1) * 128],
                                 rhs=PL[:], start=True, stop=True)
                nc.scalar.activation(out=z1[:, m * BS:(m + 1) * BS], in_=pz[:],
                                     func=AF.Relu, bias=bl1_sb[:, m:m + 1])
            z2 = const.tile([128, 2 * BS], bf16, tag="z2")
            for m in range(2):
                pz = ps.tile([128, TC], f32, name=f"pz2_{m}", tag="ps")[:, :BS]
                for kc in range(2):
                    nc.tensor.matmul(
                        out=pz[:],
                        lhsT=Wl2T[:, 256 * kc + 128 * m:256 * kc + 128 * (m + 1)],
                        rhs=z1[:, kc * BS:(kc + 1) * BS],
                        start=(kc == 0), stop=(kc == 1))
                nc.scalar.activation(out=z2[:, m * BS:(m + 1) * BS], in_=pz[:],
                                     func=AF.Relu, bias=bl2_sb[:, m:m + 1])
            y_sb = const.tile([128, 4 * BS], f32, tag="y_sb")
            for m in range(4):
                pz = ps.tile([128, TC], f32, name=f"pz3_{m}", tag="ps")[:, :BS]
                for kc in range(2):
                    nc.tensor.matmul(
                        out=pz[:],
                        lhsT=Wl3T[:, 512 * kc + 128 * m:512 * kc + 128 * (m + 1)],
                        rhs=z2[:, kc * BS:(kc + 1) * BS],
                        start=(kc == 0), stop=(kc == 1))
                nc.scalar.activation(out=y_sb[:, m * BS:(m + 1) * BS], in_=pz[:],
                                     func=AF.Identity, bias=bl3_sb[:, m:m + 1])
            nc.sync.dma_start(out=y_d[:], in_=y_sb[:])

    nc.compile()
    return nc


def _get_nc(ns_key):
    nc = _CACHE.get(ns_key)
    if nc is None:
        nc = _CACHE[ns_key] = _build(list(ns_key))
    return nc


def _host_prep(x, lengths, W1, b1, W2, b2, Wl1, bl1, Wl2, bl2, Wl3, bl3):
    bf16 = ml_dtypes.bfloat16
    x = np.asarray(x, np.float32)
    lengths = np.asarray(lengths, np.int32)
    W1 = np.asarray(W1, np.float32)      # [D,1,3,D]
    b1 = np.asarray(b1, np.float32)
    W2 = np.asarray(W2, np.float32).reshape(D, D)
    b2 = np.asarray(b2, np.float32)
    Wl1 = np.asarray(Wl1, np.float32)    # [256, D]
    bl1 = np.asarray(bl1, np.float32)
    Wl2 = np.asarray(Wl2, np.float32)
    bl2 = np.asarray(bl2, np.float32)
    Wl3 = np.asarray(Wl3, np.float32)
    bl3 = np.asarray(bl3, np.float32)

    # ---- sort by length, assign ranks: slot s <- ranks [16s, 16s+16),
    # core c gets ranks 16s+2c (pos 0 -> partitions 0:64) and 16s+2c+1.
    order = np.argsort(-lengths, kind="stable")
    Ls = lengths[order]
    ns_list = []
    for s in range(NSLOT):
        mx = int(Ls[16 * s:16 * s + 16].max())
        ns_list.append(max(1, -(-mx // TC)))
    ns_key = tuple(ns_list)

    wk = W1[:, 0]                        # [F, 3, D]; h += x[t+k] @ wk[:,k].T

    # ---- per-item device frames + exact host-side pooling correction
    # S_dev(item) = sum_{t<512n_s-2} relu(h~[t]) with x zero-padded at L.
    # corr = S_dev - S_true, pre-divided by (L-2).
    relu_b1 = np.maximum(b1, 0.0)        # [D]
    corr_sorted = np.zeros((B, D), np.float32)
    for r in range(B):
        it = order[r]
        L = int(lengths[it])
        M = TC * ns_list[r // 16]        # frames loaded for this item
        c = np.zeros(D, np.float32)
        c += max(0, M - 2 - L) * relu_b1
        if L - 2 <= M - 3:
            c += np.maximum(b1 + wk[:, 0] @ x[it, L - 2] + wk[:, 1] @ x[it, L - 1], 0.0)
        if L - 1 <= M - 3:
            c += np.maximum(b1 + wk[:, 0] @ x[it, L - 1], 0.0)
        corr_sorted[r] = c / (L - 2)

    # ---- shared (weight) inputs, host-transposed/packed
    wpk = np.zeros((128, 3 * 128), np.float32)
    for k in range(3):
        wkT = wk[:, k].T                 # [D(in), F(out)]
        wpk[0:64, 128 * k:128 * k + 64] = wkT
        wpk[64:128, 128 * k + 64:128 * (k + 1)] = wkT
    bpk = np.concatenate([b1, b1]).reshape(128, 1)

    # fold td2 into layer 1:  z1 = relu(Wl1 @ (W2 p + b2) + bl1)
    Wl1f = Wl1 @ W2                      # [256, 64]
    bl1f = Wl1 @ b2 + bl1                # [256]
    wl1 = np.ascontiguousarray(Wl1f.T)   # [64, 256]
    wl2 = np.ascontiguousarray(Wl2.T)    # [256, 256] -> [128, 2*256]
    wl2 = wl2.reshape(2, 128, 256).transpose(1, 0, 2).reshape(128, 512)
    wl3 = np.ascontiguousarray(Wl3.T)    # [256, 512] -> [128, 2*512]
    wl3 = wl3.reshape(2, 128, 512).transpose(1, 0, 2).reshape(128, 1024)
    shared = {
        "wpk": wpk.astype(bf16),
        "bpk": bpk,
        "wl1": wl1.astype(bf16),
        "wl2": np.ascontiguousarray(wl2).astype(bf16),
        "wl3": np.ascontiguousarray(wl3).astype(bf16),
        "bl1": np.ascontiguousarray(bl1f.reshape(2, 128).T),
        "bl2": np.ascontiguousarray(bl2.reshape(2, 128).T),
        "bl3": np.ascontiguousarray(bl3.reshape(4, 128).T),
    }

    # ---- per-core ragged transposed bf16 x (slot-contiguous), inv, corr
    in_maps = []
    for c in range(NCORES):
        inv = np.zeros((128, NSLOT), np.float32)
        corr = np.zeros((64, BS), np.float32)
        m = {"inv": inv, "corr": corr, **shared}
        for s in range(NSLOT):
            F_s = TC * ns_list[s]
            xt = np.zeros((128, F_s), bf16)
            for pos in range(2):
                r = 16 * s + 2 * c + pos
                it = order[r]
                L = int(lengths[it])
                n_use = min(L, F_s)
                xt[64 * pos:64 * pos + 64, :n_use] = \
                    x[it, :n_use].T.astype(bf16)
                inv[64 * pos:64 * pos + 64, s] = 1.0 / (L - 2)
                corr[:, 2 * s + pos] = corr_sorted[r]
            m[f"xt{s}"] = xt
        in_maps.append(m)

    return ns_key, in_maps, order


def _gather_out(per_core_y, order):
    # y_sb[:, m*BS + j] = feats[m*128:(m+1)*128] of device item j = 2s+pos
    # on core c  -> global rank 16s+2c+pos.
    out = np.empty((B, OUT), np.float32)
    for c in range(NCORES):
        Y = np.asarray(per_core_y[c], np.float32)   # [128, 4*BS]
        feats = Y.reshape(128, 4, BS).transpose(2, 1, 0).reshape(BS, OUT)
        for j in range(BS):
            s, pos = divmod(j, 2)
            out[order[16 * s + 2 * c + pos]] = feats[j]
    return out


def kernel(x, lengths, W1, b1, W2, b2, Wl1, bl1, Wl2, bl2, Wl3, bl3,
           _want_trace=False, **_ignored):
    from concourse.bass_utils import run_bass_kernel_spmd

    ns_key, in_maps, order = _host_prep(
        x, lengths, W1, b1, W2, b2, Wl1, bl1, Wl2, bl2, Wl3, bl3)
    nc = _get_nc(ns_key)
    res = run_bass_kernel_spmd(nc, in_maps, list(range(NCORES)),
                               trace=_want_trace)
    if _want_trace:
        _CACHE["last_result"] = res
    return _gather_out([res.results[c]["y"] for c in range(NCORES)], order)


# revision 12
# speedup vs baseline: 3.4049x; 1.0645x over previous
"""Trainium2 Bass kernel for ConvolutionFeatureProcessor.

Math (per item, matching the jax reference):
  h[t]   = relu(b1 + sum_k x[t+k] @ w1k^T)          t in [0, T-2)
  pooled = (1/(L-2)) * sum_{t<L-2} h[t]             (masked mean)
  p2     = W2 @ pooled + b2      (td2 linear -> commutes with the mean)
  out    = MLP(p2)               (64 -> 256 -> 256 -> 512)

Strategy (8 cores, data parallel over the batch, ragged-aware):
  - Host sorts items by length and packs 2 similar-length items per
    128-partition "slot"; slot s processes only ceil(Lmax_s/512) chunks
    of 512 frames (compile-time constants -> program cache key).
  - Host pre-transposes x to [d, t] layout, zero-pads each item's tail,
    and casts to bf16, so the device does a single contiguous bf16 load
    per slot (half the HBM traffic, no on-device transpose).
  - No masking on device: tails are zero-padded, and the host computes
    the exact correction (2 boundary frames + count * relu(b1)) that the
    device subtracts from the pooled sum.
  - Conv: weight-stationary k-sweeps (3 matmuls per chunk, block-diag
    w_k^T lhsT), one PSUM bank per chunk (up to 8 in flight).
  - relu+bias+pool-accum in ONE tensor_scalar op per chunk, alternating
    between the Vector and GpSimd engines.
  - td2 is folded into the first MLP layer on host; MLP runs in bf16
    feature-major; output is written [feat, item] and untangled on host.
"""

import numpy as np
import ml_dtypes

B, T, D, OUT = 128, 4096, 64, 512
NCORES = 8
BS = B // NCORES       # items per core
NSLOT = BS // 2        # pair-slots per core
TC = 512               # conv chunk (free dim per matmul / psum bank)

_CACHE = {}


def _build(ns_list, dbg=False):
    """Compile the SPMD program for per-slot chunk counts `ns_list`."""
    import concourse.bacc as bacc
    import concourse.mybir as mybir
    import concourse.tile as tile

    f32 = mybir.dt.float32
    bf16 = mybir.dt.bfloat16
    AX = mybir.AxisListType
    OP = mybir.AluOpType
    AF = mybir.ActivationFunctionType

    F_tot = sum(TC * n for n in ns_list)

    nc = bacc.Bacc("TRN2", target_bir_lowering=False, debug=False)

    xts_d = [nc.dram_tensor(f"xt{s}", [128, TC * ns_list[s]], bf16,
                             kind="ExternalInput").ap()
             for s in range(NSLOT)]
    wpk_d = nc.dram_tensor("wpk", [128, 3 * 128], bf16, kind="ExternalInput").ap()
    bpk_d = nc.dram_tensor("bpk", [128, 1], f32, kind="ExternalInput").ap()
    inv_d = nc.dram_tensor("inv", [128, NSLOT], f32, kind="ExternalInput").ap()
    corr_d = nc.dram_tensor("corr", [64, BS], f32, kind="ExternalInput").ap()
    wl1_d = nc.dram_tensor("wl1", [64, 256], bf16, kind="ExternalInput").ap()
    wl2_d = nc.dram_tensor("wl2", [128, 512], bf16, kind="ExternalInput").ap()
    wl3_d = nc.dram_tensor("wl3", [128, 1024], bf16, kind="ExternalInput").ap()
    bl1_d = nc.dram_tensor("bl1", [128, 2], f32, kind="ExternalInput").ap()
    bl2_d = nc.dram_tensor("bl2", [128, 2], f32, kind="ExternalInput").ap()
    bl3_d = nc.dram_tensor("bl3", [128, 4], f32, kind="ExternalInput").ap()
    y_d = nc.dram_tensor("y", [128, 4 * BS], f32, kind="ExternalOutput").ap()
    if dbg:
        dbg_pool = nc.dram_tensor("dbg_pool", [128, NSLOT], f32,
                                  kind="ExternalOutput").ap()
        dbg_part = nc.dram_tensor("dbg_part", [128, 8], f32,
                                  kind="ExternalOutput").ap()
        dbg_h = nc.dram_tensor("dbg_h", [128, TC], f32,
                               kind="ExternalOutput").ap()
        dbg_pl = nc.dram_tensor("dbg_pl", [64, BS], f32,
                                kind="ExternalOutput").ap()

    with tile.TileContext(nc) as tc:
        with (
            tc.tile_pool(name="const", bufs=1) as const,
            tc.tile_pool(name="xt", bufs=3) as xt_pool,
            tc.tile_pool(name="junk", bufs=1) as junk_pool,
            tc.tile_pool(name="smalls", bufs=2) as smalls,
            tc.tile_pool(name="ps", bufs=8, space="PSUM") as ps,
        ):
            # ---------------- constant loads (pure DMA) ----------------
            W_pack = const.tile([128, 3 * 128], bf16, tag="wpk")
            nc.sync.dma_start(out=W_pack[:], in_=wpk_d[:])
            b_pack = const.tile([128, 1], f32, tag="bpk")
            nc.sync.dma_start(out=b_pack[:], in_=bpk_d[:])
            inv_all = const.tile([128, NSLOT], f32, tag="inv")
            nc.sync.dma_start(out=inv_all[:], in_=inv_d[:])
            corr_sb = const.tile([64, BS], f32, tag="corr")
            nc.sync.dma_start(out=corr_sb[:], in_=corr_d[:])
            Wl1T = const.tile([64, 256], bf16, tag="wl1")
            nc.scalar.dma_start(out=Wl1T[:], in_=wl1_d[:])
            Wl2T = const.tile([128, 512], bf16, tag="wl2")
            nc.scalar.dma_start(out=Wl2T[:], in_=wl2_d[:])
            Wl3T = const.tile([128, 1024], bf16, tag="wl3")
            nc.scalar.dma_start(out=Wl3T[:], in_=wl3_d[:])
            bl1_sb = const.tile([128, 2], f32, tag="bl1")
            nc.scalar.dma_start(out=bl1_sb[:], in_=bl1_d[:])
            bl2_sb = const.tile([128, 2], f32, tag="bl2")
            nc.scalar.dma_start(out=bl2_sb[:], in_=bl2_d[:])
            bl3_sb = const.tile([128, 4], f32, tag="bl3")
            nc.scalar.dma_start(out=bl3_sb[:], in_=bl3_d[:])

            pooled_all = const.tile([128, NSLOT], f32, tag="pooled_all")
            junk_v = junk_pool.tile([128, TC], bf16, tag="junk_v")
            junk_g = junk_pool.tile([128, TC], bf16, tag="junk_g")
            zeros_sb = const.tile([128, TC], bf16, tag="zeros_sb")
            nc.vector.memset(zeros_sb[:], 0.0)

            # ---------------- per-slot streaming conv ----------------
            for s in range(NSLOT):
                n_s = ns_list[s]
                F_s = TC * n_s
                xt = xt_pool.tile([128, F_s], bf16, name=f"xt{s}", tag="xt")
                ldq = nc.gpsimd if s % 2 == 0 else nc.sync
                ldq.dma_start(out=xt[:], in_=xts_d[s][:])

                psums = [ps.tile([128, TC], f32, name=f"ps{s}_{n}", tag="ps")
                         for n in range(n_s)]
                for k in range(3):
                    lhsT = W_pack[:, 128 * k:128 * (k + 1)]
                    for n in range(n_s):
                        N = TC if n < n_s - 1 else TC - 2
                        nc.tensor.matmul(
                            out=psums[n][:, :N], lhsT=lhsT,
                            rhs=xt[:, TC * n + k:TC * n + k + N],
                            start=(k == 0), stop=(k == 2))

                partials = smalls.tile([128, n_s], f32, name=f"pt{s}",
                                       tag="partials")
                for n in range(n_s):
                    N = TC if n < n_s - 1 else TC - 2
                    if n % 8 in (2, 5, 7):
                        # scalar engine: out/accum = relu(in + bias), sum
                        nc.scalar.activation(
                            out=junk_g[:, :N], in_=psums[n][:, :N],
                            func=AF.Relu, bias=b_pack[:],
                            accum_out=partials[:, n:n + 1])
                    else:
                        # DVE: out = (in + bias) max 0; accum_out = sum(out)
                        nc.vector.scalar_tensor_tensor(
                            out=junk_v[:, :N], in0=psums[n][:, :N],
                            scalar=b_pack[:], in1=zeros_sb[:, :N],
                            op0=OP.add, op1=OP.max,
                            accum_out=partials[:, n:n + 1])
                    if dbg and s == 0 and n == 0:
                        hcp = smalls.tile([128, TC], f32, tag="hcp")
                        nc.vector.tensor_copy(hcp[:, :N], junk_v[:, :N])
                        nc.sync.dma_start(out=dbg_h[:, :N], in_=hcp[:, :N])

                pool_sum = smalls.tile([128, 1], f32, name=f"psum{s}",
                                       tag="pool_sum")
                if n_s > 1:
                    nc.vector.tensor_reduce(out=pool_sum[:], in_=partials[:],
                                            axis=AX.X, op=OP.add)
                else:
                    nc.vector.tensor_copy(pool_sum[:], partials[:])
                nc.vector.tensor_scalar(
                    out=pooled_all[:, s:s + 1], in0=pool_sum[:],
                    scalar1=inv_all[:, s:s + 1], scalar2=None, op0=OP.mult)
                if dbg and s == 0:
                    nc.sync.dma_start(out=dbg_part[:, :n_s], in_=partials[:])

            # ---------------- pooled -> MLP (bf16) ----------------
            # PL cols: item j = 2s+pos; even cols from partitions 0:64,
            # odd cols from partitions 64:128 (partition move via DMA).
            PLf = const.tile([64, BS], f32, tag="PLf")
            pv = pooled_all[:].rearrange("p (s one) -> p s one", one=1)
            plv = PLf[:].rearrange("p (s two) -> p s two", two=2)
            nc.vector.tensor_copy(plv[:, :, 0:1], pv[0:64])
            nc.sync.dma_start(out=plv[:, :, 1:2], in_=pv[64:128])
            # subtract host correction; cast to bf16
            PL = const.tile([64, BS], bf16, tag="PL")
            nc.vector.tensor_tensor(out=PL[:], in0=PLf[:], in1=corr_sb[:],
                                    op=OP.subtract)
            if dbg:
                nc.sync.dma_start(out=dbg_pool[:], in_=pooled_all[:])
                PLc = const.tile([64, BS], f32, tag="PLc")
                nc.vector.tensor_copy(PLc[:], PL[:])
                nc.sync.dma_start(out=dbg_pl[:], in_=PLc[:])

            z1 = const.tile([128, 2 * BS], bf16, tag="z1")
            for m in range(2):
                pz = ps.tile([128, TC], f32, name=f"pz1_{m}", tag="ps")[:, :BS]
                nc.tensor.matmul(out=pz[:], lhsT=Wl1T[:, m * 128:(m + 1) * 128],
                                 rhs=PL[:], start=True, stop=True)
                nc.scalar.activation(out=z1[:, m * BS:(m + 1) * BS], in_=pz[:],
                                     func=AF.Relu, bias=bl1_sb[:, m:m + 1])
            z2 = const.tile([128, 2 * BS], bf16, tag="z2")
            for m in range(2):
                pz = ps.tile([128, TC], f32, name=f"pz2_{m}", tag="ps")[:, :BS]
                for kc in range(2):
                    nc.tensor.matmul(
                        out=pz[:],
                        lhsT=Wl2T[:, 256 * kc + 128 * m:256 * kc + 128 * (m + 1)],
                        rhs=z1[:, kc * BS:(kc + 1) * BS],
                        start=(kc == 0), stop=(kc == 1))
                nc.scalar.activation(out=z2[:, m * BS:(m + 1) * BS], in_=pz[:],
                                     func=AF.Relu, bias=bl2_sb[:, m:m + 1])
            y_sb = const.tile([128, 4 * BS], f32, tag="y_sb")
            for m in range(4):
                pz = ps.tile([128, TC], f32, name=f"pz3_{m}", tag="ps")[:, :BS]
                for kc in range(2):
                    nc.tensor.matmul(
                        out=pz[:],
                        lhsT=Wl3T[:, 512 * kc + 128 * m:512 * kc + 128 * (m + 1)],
                        rhs=z2[:, kc * BS:(kc + 1) * BS],
                        start=(kc == 0), stop=(kc == 1))
                nc.scalar.activation(out=y_sb[:, m * BS:(m + 1) * BS], in_=pz[:],
                                     func=AF.Identity, bias=bl3_sb[:, m:m + 1])
            nc.sync.dma_start(out=y_d[:], in_=y_sb[:])

    nc.compile()
    return nc


def _get_nc(ns_key):
    nc = _CACHE.get(ns_key)
    if nc is None:
        nc = _CACHE[ns_key] = _build(list(ns_key))
    return nc


def _host_prep(x, lengths, W1, b1, W2, b2, Wl1, bl1, Wl2, bl2, Wl3, bl3):
    bf16 = ml_dtypes.bfloat16
    x = np.asarray(x, np.float32)
    lengths = np.asarray(lengths, np.int32)
    W1 = np.asarray(W1, np.float32)      # [D,1,3,D]
    b1 = np.asarray(b1, np.float32)
    W2 = np.asarray(W2, np.float32).reshape(D, D)
    b2 = np.asarray(b2, np.float32)
    Wl1 = np.asarray(Wl1, np.float32)    # [256, D]
    bl1 = np.asarray(bl1, np.float32)
    Wl2 = np.asarray(Wl2, np.float32)
    bl2 = np.asarray(bl2, np.float32)
    Wl3 = np.asarray(Wl3, np.float32)
    bl3 = np.asarray(bl3, np.float32)

    # ---- sort by length, assign ranks: slot s <- ranks [16s, 16s+16),
    # core c gets ranks 16s+2c (pos 0 -> partitions 0:64) and 16s+2c+1.
    order = np.argsort(-lengths, kind="stable")
    Ls = lengths[order]
    ns_list = []
    for s in range(NSLOT):
        mx = int(Ls[16 * s:16 * s + 16].max())
        ns_list.append(max(1, -(-mx // TC)))
    ns_key = tuple(ns_list)

    wk = W1[:, 0]                        # [F, 3, D]; h += x[t+k] @ wk[:,k].T

    # ---- per-item device frames + exact host-side pooling correction
    # S_dev(item) = sum_{t<512n_s-2} relu(h~[t]) with x zero-padded at L.
    # corr = S_dev - S_true, pre-divided by (L-2).
    relu_b1 = np.maximum(b1, 0.0)        # [D]
    corr_sorted = np.zeros((B, D), np.float32)
    for r in range(B):
        it = order[r]
        L = int(lengths[it])
        M = TC * ns_list[r // 16]        # frames loaded for this item
        c = np.zeros(D, np.float32)
        c += max(0, M - 2 - L) * relu_b1
        if L - 2 <= M - 3:
            c += np.maximum(b1 + wk[:, 0] @ x[it, L - 2] + wk[:, 1] @ x[it, L - 1], 0.0)
        if L - 1 <= M - 3:
            c += np.maximum(b1 + wk[:, 0] @ x[it, L - 1], 0.0)
        corr_sorted[r] = c / (L - 2)

    # ---- shared (weight) inputs, host-transposed/packed
    wpk = np.zeros((128, 3 * 128), np.float32)
    for k in range(3):
        wkT = wk[:, k].T                 # [D(in), F(out)]
        wpk[0:64, 128 * k:128 * k + 64] = wkT
        wpk[64:128, 128 * k + 64:128 * (k + 1)] = wkT
    bpk = np.concatenate([b1, b1]).reshape(128, 1)

    # fold td2 into layer 1:  z1 = relu(Wl1 @ (W2 p + b2) + bl1)
    Wl1f = Wl1 @ W2                      # [256, 64]
    bl1f = Wl1 @ b2 + bl1                # [256]
    wl1 = np.ascontiguousarray(Wl1f.T)   # [64, 256]
    wl2 = np.ascontiguousarray(Wl2.T)    # [256, 256] -> [128, 2*256]
    wl2 = wl2.reshape(2, 128, 256).transpose(1, 0, 2).reshape(128, 512)
    wl3 = np.ascontiguousarray(Wl3.T)    # [256, 512] -> [128, 2*512]
    wl3 = wl3.reshape(2, 128, 512).transpose(1, 0, 2).reshape(128, 1024)
    shared = {
        "wpk": wpk.astype(bf16),
        "bpk": bpk,
        "wl1": wl1.astype(bf16),
        "wl2": np.ascontiguousarray(wl2).astype(bf16),
        "wl3": np.ascontiguousarray(wl3).astype(bf16),
        "bl1": np.ascontiguousarray(bl1f.reshape(2, 128).T),
        "bl2": np.ascontiguousarray(bl2.reshape(2, 128).T),
        "bl3": np.ascontiguousarray(bl3.reshape(4, 128).T),
    }

    # ---- per-core ragged transposed bf16 x (slot-contiguous), inv, corr
    in_maps = []
    for c in range(NCORES):
        inv = np.zeros((128, NSLOT), np.float32)
        corr = np.zeros((64, BS), np.float32)
        m = {"inv": inv, "corr": corr, **shared}
        for s in range(NSLOT):
            F_s = TC * ns_list[s]
            xt = np.zeros((128, F_s), bf16)
            for pos in range(2):
                r = 16 * s + 2 * c + pos
                it = order[r]
                L = int(lengths[it])
                n_use = min(L, F_s)
                xt[64 * pos:64 * pos + 64, :n_use] = \
                    x[it, :n_use].T.astype(bf16)
                inv[64 * pos:64 * pos + 64, s] = 1.0 / (L - 2)
                corr[:, 2 * s + pos] = corr_sorted[r]
            m[f"xt{s}"] = xt
        in_maps.append(m)

    return ns_key, in_maps, order


def _gather_out(per_core_y, order):
    # y_sb[:, m*BS + j] = feats[m*128:(m+1)*128] of device item j = 2s+pos
    # on core c  -> global rank 16s+2c+pos.
    out = np.empty((B, OUT), np.float32)
    for c in range(NCORES):
        Y = np.asarray(per_core_y[c], np.float32)   # [128, 4*BS]
        feats = Y.reshape(128, 4, BS).transpose(2, 1, 0).reshape(BS, OUT)
        for j in range(BS):
            s, pos = divmod(j, 2)
            out[order[16 * s + 2 * c + pos]] = feats[j]
    return out


def kernel(x, lengths, W1, b1, W2, b2, Wl1, bl1, Wl2, bl2, Wl3, bl3,
           _want_trace=False, **_ignored):
    from concourse.bass_utils import run_bass_kernel_spmd

    ns_key, in_maps, order = _host_prep(
        x, lengths, W1, b1, W2, b2, Wl1, bl1, Wl2, bl2, Wl3, bl3)
    nc = _get_nc(ns_key)
    res = run_bass_kernel_spmd(nc, in_maps, list(range(NCORES)),
                               trace=_want_trace)
    if _want_trace:
        _CACHE["last_result"] = res
    return _gather_out([res.results[c]["y"] for c in range(NCORES)], order)
